# revision 18
# baseline (speedup 1.0000x reference)
"""MinGRU (GRU-style gated recurrence) Trainium2 Bass kernel.

Problem: x [64, 1024, 512], Wz/Wh [512, 1024], bz/bh [512]
    h_t = (1-z_t)*h_{t-1} + z_t*htilde_t
    z_t = sigmoid([x_t, h_{t-1}] @ Wz.T + bz)
    htilde_t = tanh([x_t, h_{t-1}] @ Wh.T + bh)
Returns (outputs [64, 1024, 512], h_last [64, 512]).

Sharding: data-parallel, batch 64 -> 8 cores x 8 rows. Hidden state is
device-local across the scan; weights replicated.

Per-core structure (see git-less history in comments):
  - One-time: transpose Wz/Wh on the PE into [k, gate] layout; bf16 copy
    for the recurrent part, fp32r for the input part.
  - Per 16-step chunk: precompute A = x_t @ Wx.T + b for 16 steps as one
    M=128 matmul block (fp32r); cast A to bf16.
  - Scan (sequential): per step
      * inject A_t + bias into PSUM via a one-hot column-select matmul
      * 8 bf16 matmuls (4 k-tiles x 2 gates) for the h-part; h is kept
        transposed (hT, [128, 4*8]) so it is the stationary operand
      * sigmoid/tanh on ScalarE (bf16 out)
      * PE-transpose z/htilde into hT layout ([128, 8] tiles)
      * combine on VectorE in [128, 32] layout:
          d = htT - hT; m = zT*d; hT_bf16 = hT + m (chain);
          stage_f32 = hT + m (off-chain, feeds output + next combine)
  - Output: per chunk, PE-transpose the fp32 stage back to row-major
    [(b,t,k), h128] blocks and DMA with per-partition contiguous runs.
"""

import sys

if "/opt/trn_rl_repo" not in sys.path:
    sys.path.insert(0, "/opt/trn_rl_repo")

import numpy as np

import concourse.bacc as bacc
import concourse.mybir as mybir
import concourse.tile as tile
from concourse import bass_utils

F32 = mybir.dt.float32
F32R = mybir.dt.float32r
BF16 = mybir.dt.bfloat16
AF = mybir.ActivationFunctionType

B, S, I, H = 64, 1024, 512, 512
NCORES = 8
BL = B // NCORES  # 8 batch rows per core
CH = 16           # timesteps per chunk (16*8 = 128 rows per precompute block)
G2 = 2 * H        # 1024 gate columns, [z | h]
KT = H // 128     # 4 k-tiles over the hidden dim


def build(s_len=S, ch=CH):
    nch = s_len // ch
    assert nch * ch == s_len
    rows = ch * BL
    assert rows == 128
    CW = ch * 4  # stage columns per batch row (t,k)

    nc = bacc.Bacc(None, target_bir_lowering=False, debug=False)

    x = nc.dram_tensor("x", [BL, s_len, I], F32R, kind="ExternalInput")
    wz = nc.dram_tensor("wz", [H, I + H], F32R, kind="ExternalInput")
    wh = nc.dram_tensor("wh", [H, I + H], F32R, kind="ExternalInput")
    bzv = nc.dram_tensor("bz", [1, H], F32R, kind="ExternalInput")
    bhv = nc.dram_tensor("bh", [1, H], F32R, kind="ExternalInput")
    ident = nc.dram_tensor("ident", [128, 128], F32R, kind="ExternalInput")
    identb = nc.dram_tensor("identb", [128, 128], BF16, kind="ExternalInput")
    h0d = nc.dram_tensor("h0", [128, 4 * BL], F32R, kind="ExternalInput")
    h0bd = nc.dram_tensor("h0b", [128, 4 * BL], BF16, kind="ExternalInput")
    out = nc.dram_tensor("out", [BL, s_len, H], F32R, kind="ExternalOutput")

    with tile.TileContext(nc) as tc:
        with (
            tc.tile_pool(name="const", bufs=1) as constp,
            tc.tile_pool(name="winit", bufs=1) as winitp,
            tc.tile_pool(name="xin", bufs=2) as xinp,
            tc.tile_pool(name="xts", bufs=2) as xtsp,
            tc.tile_pool(name="asb", bufs=2) as asbp,
            tc.tile_pool(name="acts", bufs=2) as actsp,
            tc.tile_pool(name="comb", bufs=2) as combp,
            tc.tile_pool(name="hbf", bufs=2) as hbfp,
            tc.tile_pool(name="onat", bufs=2) as onatp,
            tc.tile_pool(name="stage", bufs=2) as stagep,
            tc.tile_pool(name="pz", bufs=2, space="PSUM") as pzp,
            tc.tile_pool(name="ph", bufs=2, space="PSUM") as php,
            tc.tile_pool(name="pt", bufs=2, space="PSUM") as ptp,
            tc.tile_pool(name="pa", bufs=1, space="PSUM") as pap,
        ):
            # ---------------- constants ----------------
            ident_sb = constp.tile([128, 128], F32R, tag="ident")
            nc.sync.dma_start(out=ident_sb, in_=ident[:, :])
            identb_sb = constp.tile([128, 128], BF16, tag="identb")
            nc.sync.dma_start(out=identb_sb, in_=identb[:, :])

            bias_sb = constp.tile([1, G2], F32R, tag="bias")
            nc.sync.dma_start(out=bias_sb[:, 0:H], in_=bzv[:, :])
            nc.sync.dma_start(out=bias_sb[:, H:G2], in_=bhv[:, :])
            onesd = nc.dram_tensor("ones", [1, 128], F32R, kind="ExternalInput")
            onesd_sb = constp.tile([1, 128], F32R, tag="ones")
            nc.sync.dma_start(out=onesd_sb, in_=onesd[:, :])

            hT0 = constp.tile([128, 4 * BL], F32R, tag="h0")
            nc.sync.dma_start(out=hT0, in_=h0d[:, :])
            hT0b = constp.tile([128, 4 * BL], BF16, tag="h0b")
            nc.sync.dma_start(out=hT0b, in_=h0bd[:, :])

            # one-hot column selector for the inject matmul (bf16)
            identb_sel = identb_sb.rearrange("p (b t) -> p t b", t=ch)

            # ---------------- weights: load + transpose ----------------
            # wxT[p, k*G2 + g] = W'[g, 128k+p] for the x-part (fp32r);
            # wrTb same for the h-part, cast to bf16.
            wxT = constp.tile([128, KT * G2], F32R, tag="wxT")
            wrTb = constp.tile([128, KT * G2], BF16, tag="wrTb")

            for mi, wsrc in ((0, wz), (1, wh)):
                wnat = winitp.tile([128, 4 * (I + H)], F32R, tag=f"wnat{mi}")
                nc.sync.dma_start(
                    out=wnat.rearrange("p (r c) -> p r c", r=4),
                    in_=wsrc.rearrange("(r p) c -> p r c", p=128),
                )
                # wnat[p, r*1024 + c] = W[128r+p, c]
                for cc in range(8):  # column block of W (k-dim): cols 128cc..
                    tp = ptp.tile([128, 512], F32R, tag="tps")
                    for r in range(4):  # gate-row block
                        nc.tensor.transpose(
                            out=tp[:, 128 * r : 128 * (r + 1)],
                            in_=wnat[:, 1024 * r + 128 * cc : 1024 * r + 128 * (cc + 1)],
                            identity=ident_sb,
                        )
                    # tp[p2, 128r + p] = W[128r+p, 128cc+p2]
                    if cc < 4:
                        dst = wxT[:, cc * G2 + mi * H : cc * G2 + mi * H + 512]
                    else:
                        k = cc - 4
                        dst = wrTb[:, k * G2 + mi * H : k * G2 + mi * H + 512]
                    nc.vector.tensor_copy(dst, tp)

            # helper: AP over a hT holder enumerating (k outer, b inner)
            def kb_view(tile_ap, tt=None):
                if tt is None:  # hT0-style [128, (k b)]
                    return tile_ap.rearrange("p (k b) -> p k b", k=4)
                return tile_ap.rearrange("p (b t k) -> p t k b", b=BL, k=4)[:, tt]

            # ---------------- main loop ----------------
            hT_prev = kb_view(hT0)      # fp32 state, (k,b) enumeration
            hbf_prev = hT0b             # bf16 state, cols = 8k + b
            for c in range(nch):
                t0 = c * ch

                # ---- precompute A for this chunk (fp32r matmuls) ----
                # rows of the block are (b, t) b-major: row = b*ch + t
                x_sb = xinp.tile([128, I], F32R, tag="x")
                nc.sync.dma_start(out=x_sb[:, :], in_=x[:, t0 : t0 + ch, :])
                xt_ps = ptp.tile([128, 512], F32R, tag="tps")
                for k in range(4):
                    nc.tensor.transpose(
                        out=xt_ps[:, 128 * k : 128 * (k + 1)],
                        in_=x_sb[:, 128 * k : 128 * (k + 1)],
                        identity=ident_sb,
                    )
                xt_sb = xtsp.tile([128, 512], F32R, tag="xt")
                nc.vector.tensor_copy(xt_sb, xt_ps)

                a_ps = pap.tile([128, G2], F32, tag="aps")
                for hf in range(2):
                    o = a_ps[:, 512 * hf : 512 * (hf + 1)]
                    nc.tensor.matmul(
                        o,
                        lhsT=onesd_sb,
                        rhs=bias_sb[:, 512 * hf : 512 * (hf + 1)],
                        start=True,
                        stop=False,
                    )
                    for k in range(4):
                        nc.tensor.matmul(
                            o,
                            lhsT=xt_sb[:, 128 * k : 128 * (k + 1)],
                            rhs=wxT[:, k * G2 + 512 * hf : k * G2 + 512 * (hf + 1)],
                            start=False,
                            stop=(k == 3),
                        )
                a_sb = asbp.tile([128, G2], BF16, tag="a")
                nc.vector.tensor_copy(a_sb, a_ps)

                # fp32 hT staging for this chunk; col = b*CW + t*4 + k
                stage = stagep.tile([128, ch * 4 * BL], F32R, tag="stage")

                # ---- scan ----
                for tt in range(ch):
                    z_ps = pzp.tile([BL, 512], F32, tag="zps")
                    h_ps = php.tile([BL, 512], F32, tag="hps")
                    lhsE = identb_sel[:, tt, :]  # selects rows b*ch + tt
                    nc.tensor.matmul(
                        z_ps, lhsT=lhsE, rhs=a_sb[:, 0:512],
                        start=True, stop=False,
                    )
                    nc.tensor.matmul(
                        h_ps, lhsT=lhsE, rhs=a_sb[:, 512:1024],
                        start=True, stop=False,
                    )
                    for k in range(4):
                        lh = hbf_prev[:, 8 * k : 8 * k + 8]
                        nc.tensor.matmul(
                            z_ps, lhsT=lh,
                            rhs=wrTb[:, k * G2 : k * G2 + 512],
                            start=False, stop=(k == 3),
                        )
                        nc.tensor.matmul(
                            h_ps, lhsT=lh,
                            rhs=wrTb[:, k * G2 + 512 : k * G2 + 1024],
                            start=False, stop=(k == 3),
                        )

                    hb_sb = actsp.tile([BL, 512], BF16, tag="hb")
                    z_sb = actsp.tile([BL, 512], BF16, tag="z")
                    nc.scalar.activation(hb_sb, h_ps, AF.Tanh)
                    nc.scalar.activation(z_sb, z_ps, AF.Sigmoid)

                    t_ps = ptp.tile([128, 512], BF16, tag="tps")
                    for k in range(4):
                        nc.tensor.transpose(
                            out=t_ps[:, 8 * k : 8 * k + 8],
                            in_=hb_sb[:, 128 * k : 128 * (k + 1)],
                            identity=identb_sb[0:BL, 0:BL],
                        )
                    for k in range(4):
                        nc.tensor.transpose(
                            out=t_ps[:, 32 + 8 * k : 32 + 8 * k + 8],
                            in_=z_sb[:, 128 * k : 128 * (k + 1)],
                            identity=identb_sb[0:BL, 0:BL],
                        )

                    d_sb = combp.tile([128, 32], F32, tag="d")
                    m_sb = combp.tile([128, 32], F32, tag="m")
                    hbf = hbfp.tile([128, 32], BF16, tag="hbf")
                    # (k,b) enumeration everywhere
                    tps_hb = t_ps[:, 0:32].rearrange("p (k b) -> p k b", k=4)
                    tps_z = t_ps[:, 32:64].rearrange("p (k b) -> p k b", k=4)
                    d_v = d_sb.rearrange("p (k b) -> p k b", k=4)
                    m_v = m_sb.rearrange("p (k b) -> p k b", k=4)
                    hbf_v = hbf.rearrange("p (k b) -> p k b", k=4)
                    hT_new = kb_view(stage, tt)
                    nc.vector.tensor_sub(d_v, tps_hb, hT_prev)
                    nc.vector.tensor_mul(m_v, tps_z, d_v)
                    nc.vector.tensor_add(hbf_v, hT_prev, m_v)   # chain: bf16
                    nc.vector.tensor_add(hT_new, hT_prev, m_v)  # off-chain f32
                    hT_prev = hT_new
                    hbf_prev = hbf

                # ---- write chunk output ----
                # transpose stage back to row-major blocks and DMA with
                # per-partition contiguous 512B runs.
                onat_ps = ptp.tile([128, 512], F32R, tag="tps")
                for j in range(4):
                    nc.tensor.transpose(
                        out=onat_ps[:, 128 * j : 128 * (j + 1)],
                        in_=stage[:, 128 * j : 128 * (j + 1)],
                        identity=ident_sb,
                    )
                out_nat = onatp.tile([128, 512], F32R, tag="onat")
                nc.vector.tensor_copy(out_nat, onat_ps)
                out_v = out.rearrange("b s (k p) -> b (s k) p", p=128)
                for j in range(4):
                    nc.sync.dma_start(
                        out=out_v[2 * j : 2 * j + 2, 4 * t0 : 4 * t0 + CW, :],
                        in_=out_nat[:, 128 * j : 128 * (j + 1)],
                    )

    nc.compile()
    return nc


_CACHE = {}


def _get_nc(s_len=S, ch=CH):
    key = (s_len, ch)
    if key not in _CACHE:
        _CACHE[key] = build(s_len, ch)
    return _CACHE[key]


def make_in_maps(inputs):
    import ml_dtypes

    x = np.ascontiguousarray(np.asarray(inputs["x"], dtype=np.float32))
    wz = np.ascontiguousarray(np.asarray(inputs["Wz"], dtype=np.float32))
    wh = np.ascontiguousarray(np.asarray(inputs["Wh"], dtype=np.float32))
    bz = np.asarray(inputs["bz"], dtype=np.float32).reshape(1, H)
    bh = np.asarray(inputs["bh"], dtype=np.float32).reshape(1, H)
    ident = np.eye(128, dtype=np.float32)
    identb = np.eye(128, dtype=ml_dtypes.bfloat16)
    return [
        {
            "x": np.ascontiguousarray(x[c * BL : (c + 1) * BL]),
            "wz": wz,
            "wh": wh,
            "bz": bz,
            "bh": bh,
            "ident": ident,
            "identb": identb,
            "ones": np.ones((1, 128), dtype=np.float32),
            "h0": np.zeros((128, 4 * BL), dtype=np.float32),
            "h0b": np.zeros((128, 4 * BL), dtype=ml_dtypes.bfloat16),
        }
        for c in range(NCORES)
    ]


def kernel(**inputs):
    nc = _get_nc()
    in_maps = make_in_maps(inputs)
    res = bass_utils.run_bass_kernel_spmd(nc, in_maps, core_ids=list(range(NCORES)))
    outs = np.concatenate([res.results[c]["out"] for c in range(NCORES)], axis=0)
    return outs, np.ascontiguousarray(outs[:, -1, :])
